# revision 1
# baseline (speedup 1.0000x reference)
"""Trainium2 Bass kernel for the LIIF-style guided upsampling MLP (nn_BF_NIR_conv).

Key structural insight: `grid_sample(nearest)` at the 4 shifted coords reduces to
parity-dependent integer shifts of the LR grid — for HR pixel (2k+p, 2l+q) and
neighbor (vx,vy)=(2a-1,2b-1), the sampled LR position is (k+p-1+a, l+q-1+b).
So we process pixels grouped by parity class (p,q); every "gather" becomes a
contiguous shifted window over a zero-padded LR feature slice, and `rel` is a
per-(class,neighbor) constant folded into the layer-1 bias (with small additive
fixup tiles for image-border pixels, where the reference's joint-validity rule
makes rel pixel-dependent).

Bilateral softmax weights: gc values for all 16 (class, neighbor) combos are
exactly the 9 LR-level shifted 3-channel dot-product maps D[dy,dx]; partition
reductions are done with tiny 0/1 selection matmuls on the PE.

Sharding: core c handles HR rows [32c, 32c+32) — data-parallel over pixels, with
an 18-row LR halo slice instead of full replication.
"""
import numpy as np

import concourse.bass as bass
import concourse.tile as tile
from concourse import mybir, bacc
from concourse.bass_utils import run_bass_kernel_spmd

F32 = mybir.dt.float32
AF = mybir.ActivationFunctionType
ALU = mybir.AluOpType
F32R = mybir.dt.float32r


def _r(ap):
    return ap.bitcast(F32R)

NCORES = 8
# combos enumerated as cmb = (2p+q)*4 + (2a+b)
ALL16 = [(p, q, a, b) for p in (0, 1) for q in (0, 1) for a in (0, 1) for b in (0, 1)]
ALL16 = sorted(ALL16, key=lambda t: ((2 * t[0] + t[1]) * 4 + 2 * t[2] + t[3]))
# col-border combos (l=0 col invalid for b=0&q=0; l=127 col invalid for b=1&q=1)
CB = [t for t in ALL16 if (t[1] == 0 and t[3] == 0) or (t[1] == 1 and t[3] == 1)]

_NC = None


def _build_nc():
    global _NC
    if _NC is not None:
        return _NC
    nc = bacc.Bacc("TRN2", target_bir_lowering=False)

    fc0 = nc.dram_tensor("fc0", [128, 18 * 130], F32R, kind="ExternalInput")
    fc1 = nc.dram_tensor("fc1", [128, 18 * 130], F32R, kind="ExternalInput")
    guide = nc.dram_tensor("guide", [128, 4 * 2048], F32R, kind="ExternalInput")
    w1 = nc.dram_tensor("w1", [128, 3 * 256], F32R, kind="ExternalInput")
    w2 = nc.dram_tensor("w2", [128, 2 * 128], F32R, kind="ExternalInput")
    w3 = nc.dram_tensor("w3", [128, 32], F32R, kind="ExternalInput")
    bias1 = nc.dram_tensor("bias1", [128, 32], F32, kind="ExternalInput")
    b2t = nc.dram_tensor("b2", [128, 1], F32, kind="ExternalInput")
    b3t = nc.dram_tensor("b3", [128, 1], F32, kind="ExternalInput")
    colfix = nc.dram_tensor("colfix", [128, 256], F32, kind="ExternalInput")
    rowfix = nc.dram_tensor("rowfix", [128, 2048], F32, kind="ExternalInput")
    selS = nc.dram_tensor("selS", [27, 9], F32, kind="ExternalInput")
    selB = nc.dram_tensor("selB", [9, 4], F32, kind="ExternalInput")
    selR = nc.dram_tensor("selR", [128, 32], F32, kind="ExternalInput")
    # class-grouped output [32ch, cls, k, l] — contiguous stores; host de-interleaves
    y = nc.dram_tensor("y", [32, 4 * 2048], F32, kind="ExternalOutput")

    with tile.TileContext(nc) as tc, \
         tc.tile_pool(name="const", bufs=1) as constp, \
         tc.tile_pool(name="gpool", bufs=2) as gpool, \
         tc.tile_pool(name="pipe", bufs=1) as pipe, \
         tc.tile_pool(name="work", bufs=3) as workp, \
         tc.tile_pool(name="ph1", bufs=2, space="PSUM") as ph1, \
         tc.tile_pool(name="ph2", bufs=2, space="PSUM") as ph2, \
         tc.tile_pool(name="ppred", bufs=1, space="PSUM") as ppred, \
         tc.tile_pool(name="pout", bufs=1, space="PSUM") as pout:

        # ---- constants in ----
        s_fc0 = constp.tile([128, 18 * 130], F32R)
        s_fc1 = constp.tile([128, 18 * 130], F32R)
        nc.sync.dma_start(out=s_fc0, in_=fc0[:, :])
        nc.sync.dma_start(out=s_fc1, in_=fc1[:, :])
        s_w1 = constp.tile([128, 3 * 256], F32R)
        nc.sync.dma_start(out=s_w1, in_=w1[:, :])
        s_w2 = constp.tile([128, 2 * 128], F32R)
        nc.sync.dma_start(out=s_w2, in_=w2[:, :])
        s_w3 = constp.tile([128, 32], F32R)
        nc.sync.dma_start(out=s_w3, in_=w3[:, :])
        s_bias1 = constp.tile([128, 32], F32)
        nc.sync.dma_start(out=s_bias1, in_=bias1[:, :])
        s_b2 = constp.tile([128, 1], F32)
        nc.sync.dma_start(out=s_b2, in_=b2t[:, :])
        s_b3 = constp.tile([128, 1], F32)
        nc.sync.dma_start(out=s_b3, in_=b3t[:, :])
        s_colfix = constp.tile([128, 256], F32)
        nc.sync.dma_start(out=s_colfix, in_=colfix[:, :])
        s_rowfix = constp.tile([128, 2048], F32)
        nc.sync.dma_start(out=s_rowfix, in_=rowfix[:, :])
        s_S = constp.tile([27, 9], F32)
        nc.sync.dma_start(out=s_S, in_=selS[:, :])
        s_B = constp.tile([9, 4], F32)
        nc.sync.dma_start(out=s_B, in_=selB[:, :])
        s_R = constp.tile([128, 32], F32)
        nc.sync.dma_start(out=s_R, in_=selR[:, :])

        fc0r = s_fc0[:, :].rearrange("c (r x) -> c r x", x=130)
        fc1r = s_fc1[:, :].rearrange("c (r x) -> c r x", x=130)

        # ---- bilateral weight pipeline (LR-indexed) ----
        # Unnormalized weights: W128 holds broadcast exp(D); the softmax
        # denominator 1/s is applied at the output stage via R32.
        # T1 = center 3-channel features replicated 9x; T2 = the 9 shifted windows
        dma_engs = [nc.scalar, nc.gpsimd, nc.sync]
        T1 = pipe.tile([27, 2048], F32, tag="tA")
        T2 = pipe.tile([27, 2048], F32, tag="tB")
        for g in range(9):
            u, v = divmod(g, 3)
            dma_engs[g % 3].dma_start(
                out=T2[3 * g:3 * g + 3, :].rearrange("c (r x) -> c r x", x=128),
                in_=fc1r[124:127, u:u + 16, v:v + 128].bitcast(F32))
            dma_engs[(g + 1) % 3].dma_start(
                out=T1[3 * g:3 * g + 3, :].rearrange("c (r x) -> c r x", x=128),
                in_=fc1r[124:127, 1:17, 1:129].bitcast(F32))
        nc.vector.tensor_mul(T1[:, :], T1[:, :], T2[:, :])  # in place
        e_t = pipe.tile([9, 2048], F32, tag="tC")
        r_t = pipe.tile([4, 2048], F32, tag="tD")
        with tc.tile_pool(name="pwt", bufs=1, space="PSUM") as pwt:
            for ckw in range(4):
                Dp = pwt.tile([9, 512], F32, tag="wps")
                nc.tensor.matmul(Dp[:, :], s_S[:, :],
                                 T1[:, 512 * ckw:512 * (ckw + 1)],
                                 start=True, stop=True)
                nc.scalar.activation(e_t[:, 512 * ckw:512 * (ckw + 1)], Dp[:, :],
                                     AF.Exp)
            for ckw in range(4):
                sp = pwt.tile([4, 512], F32, tag="wps")
                nc.tensor.matmul(sp[:, :], s_B[:, :],
                                 e_t[:, 512 * ckw:512 * (ckw + 1)],
                                 start=True, stop=True)
                nc.vector.reciprocal_approx_fast(
                    out=r_t[:, 512 * ckw:512 * (ckw + 1)], in_=sp[:, :])
        edram = nc.dram_tensor("edram", [9, 2048], F32)
        nc.sync.dma_start(out=edram[:, :], in_=e_t[:, :])
        rdram = nc.dram_tensor("rdram", [4, 2048], F32)
        nc.sync.dma_start(out=rdram[:, :], in_=r_t[:, :])
        # broadcast rows across partitions (DRAM reads allow partition-step-0)
        W128 = [constp.tile([128, 2048], F32, tag=f"W128_{cls}", name=f"W128_{cls}")
                 for cls in range(4)]
        R32 = [constp.tile([32, 2048], F32, tag=f"R32_{cls}", name=f"R32_{cls}")
               for cls in range(4)]
        for cmb, (p, q, a, b) in enumerate(ALL16):
            g = 3 * (p + a) + (q + b)
            cls, j = 2 * p + q, 2 * a + b
            bcast = bass.AP(tensor=edram[:, :].tensor, offset=g * 2048,
                            ap=[[0, 32], [1, 2048]])
            dma_engs[cmb % 3].dma_start(out=W128[cls][32 * j:32 * j + 32, :],
                                        in_=bcast)
        for cls in range(4):
            bcast = bass.AP(tensor=rdram[:, :].tensor, offset=cls * 2048,
                            ap=[[0, 32], [1, 2048]])
            dma_engs[cls % 3].dma_start(out=R32[cls][:, :], in_=bcast)

        # ---- main per-(class, chunk) pipeline ----
        for cls in range(4):
            p, q = cls >> 1, cls & 1
            s_gd = gpool.tile([128, 2048], F32R, tag="gd")
            nc.sync.dma_start(out=s_gd, in_=guide[:, 2048 * cls:2048 * (cls + 1)])
            for ck in range(4):
                pw = workp.tile([128, 512], F32, tag="pw")
                for j in range(4):
                    a, b = j >> 1, j & 1
                    cmb = cls * 4 + j
                    h1ps = [ph1.tile([128, 512], F32, tag=f"h1ps{blk}",
                                      name=f"h1ps{blk}", bufs=(2 if blk == 0 else 1))
                            for blk in range(2)]
                    for blk in range(2):
                        ps = h1ps[blk][:, :]
                        lw = lambda kb: s_w1[:, kb * 256 + blk * 128:
                                             kb * 256 + blk * 128 + 128]
                        nc.tensor.matmul(ps, _r(lw(2)), _r(s_gd[:, 512 * ck:512 * (ck + 1)]),
                                         start=True, stop=False)
                        rs, cs = 4 * ck + p + a, q + b
                        nc.tensor.matmul(ps, _r(lw(0)), _r(fc0r[:, rs:rs + 4, cs:cs + 128]),
                                         start=False, stop=False)
                        nc.tensor.matmul(ps, _r(lw(1)), _r(fc1r[:, rs:rs + 4, cs:cs + 128]),
                                         start=False, stop=True)
                    # border fixups (pre-relu)
                    if (q == 0 and b == 0) or (q == 1 and b == 1):
                        ci = CB.index((p, q, a, b))
                        l0 = 0 if q == 0 else 127
                        for blk in range(2):
                            view = h1ps[blk][:, l0::128]
                            fx = s_colfix[:, (ci * 2 + blk) * 16 + 4 * ck:
                                          (ci * 2 + blk) * 16 + 4 * ck + 4]
                            nc.vector.tensor_add(view, view, fx)
                    if (p, a) == (0, 0) and ck == 0:
                        ri = 2 * q + b
                        for blk in range(2):
                            view = h1ps[blk][:, 0:128]
                            base = ((0 * 4 + ri) * 2 + blk) * 128
                            nc.vector.tensor_add(view, view,
                                                 s_rowfix[:, base:base + 128])
                    if (p, a) == (1, 1) and ck == 3:
                        ri = 2 * q + b
                        for blk in range(2):
                            view = h1ps[blk][:, 384:512]
                            base = ((1 * 4 + ri) * 2 + blk) * 128
                            nc.vector.tensor_add(view, view,
                                                 s_rowfix[:, base:base + 128])
                    # relu + bias -> SBUF (split across ACT and DVE)
                    h1sb = [workp.tile([128, 512], F32R, tag=f"h1sb{blk}",
                                        name=f"h1sb{blk}")
                            for blk in range(2)]
                    nc.scalar.activation(h1sb[0][:, :], h1ps[0][:, :], AF.Relu,
                                         bias=s_bias1[:, cmb * 2:cmb * 2 + 1])
                    nc.vector.tensor_scalar(h1sb[1][:, :], h1ps[1][:, :],
                                            s_bias1[:, cmb * 2 + 1:cmb * 2 + 2],
                                            0.0, ALU.add, ALU.max)
                    # layer 2
                    h2ps = ph2.tile([128, 512], F32, tag="h2ps")
                    nc.tensor.matmul(h2ps[:, :], _r(s_w2[:, 0:128]), _r(h1sb[0][:, :]),
                                     start=True, stop=False)
                    nc.tensor.matmul(h2ps[:, :], _r(s_w2[:, 128:256]), _r(h1sb[1][:, :]),
                                     start=False, stop=True)
                    h2sb = workp.tile([128, 512], F32R, tag="h2sb")
                    nc.scalar.activation(h2sb[:, :], h2ps[:, :], AF.Relu,
                                         bias=s_b2[:, 0:1])
                    # layer 3: per-neighbor [32, 512] psum (fp32r needs quadrant 0)
                    pred = ppred.tile([32, 512], F32, tag="pred")
                    nc.tensor.matmul(pred[:, :], _r(s_w3[:, 0:32]),
                                     h2sb[:, :], start=True, stop=True)
                    # weight by softmax factor, write into stacked pw slice
                    nc.vector.tensor_mul(pw[32 * j:32 * j + 32, :], pred[:, :],
                                         W128[cls][32 * j:32 * j + 32,
                                                   512 * ck:512 * (ck + 1)])
                # weighted combine, then normalize by 1/s and add b3
                ops = pout.tile([32, 512], F32, tag="ops")
                nc.tensor.matmul(ops[:, :], s_R[:, 0:32], pw[:, :],
                                 start=True, stop=True)
                osb = workp.tile([32, 512], F32, tag="osb")
                nc.vector.tensor_mul(osb[:, :], ops[:, :],
                                     R32[cls][:, 512 * ck:512 * (ck + 1)])
                nc.scalar.activation(osb[:, :], osb[:, :], AF.Identity,
                                     bias=s_b3[0:32, 0:1])
                nc.sync.dma_start(
                    out=y[:, 2048 * cls + 512 * ck:2048 * cls + 512 * (ck + 1)],
                    in_=osb[:, :])

    nc.compile()
    _NC = nc
    return nc


def _prep_core(c, feat, lr_guide, hr_guide, W1, b1, W2, b2, W3, b3):
    def pad_slice(img):  # [128, 128, 128] -> [128, 18, 130] zero-padded halo
        out = np.zeros((128, 18, 130), np.float32)
        y0 = 16 * c - 1
        ys, ye = max(y0, 0), min(16 * c + 17, 128)
        out[:, ys - y0:ye - y0, 1:129] = img[:, ys:ye, :]
        return out.reshape(128, 18 * 130)

    fc0 = pad_slice(lr_guide[0])
    fc1 = pad_slice(feat[0])
    strip = hr_guide[0][:, 32 * c:32 * c + 32, :]
    g = np.empty((128, 4, 16, 128), np.float32)
    for p in range(2):
        for q in range(2):
            g[:, 2 * p + q] = strip[:, p::2, q::2]

    W1y, W1x = W1[384], W1[385]
    bias1 = np.zeros((128, 32), np.float32)
    for cmb, (p, q, a, b) in enumerate(ALL16):
        v = b1 + (1.5 - p - 2 * a) * W1y + (1.5 - q - 2 * b) * W1x
        bias1[:, cmb * 2] = v[:128]
        bias1[:, cmb * 2 + 1] = v[128:]

    colfix = np.zeros((128, 256), np.float32)
    for ci, (p, q, a, b) in enumerate(CB):
        l0 = 0 if q == 0 else 127
        relx_inv = (2 * l0 + q) + 0.5 - 128.0
        relx_int = 1.5 - q - 2 * b
        rely_int = 1.5 - p - 2 * a
        for k in range(16):
            I = 32 * c + 2 * k + p
            d = (I + 0.5 - 128.0 - rely_int) * W1y + (relx_inv - relx_int) * W1x
            if c == 0 and (p, a) == (0, 0) and k == 0:
                d = 0 * d
            if c == 7 and (p, a) == (1, 1) and k == 15:
                d = 0 * d
            colfix[:, (ci * 2 + 0) * 16 + k] = d[:128]
            colfix[:, (ci * 2 + 1) * 16 + k] = d[128:]

    rowfix = np.zeros((128, 2048), np.float32)
    for pat in range(2):
        if (pat == 0 and c != 0) or (pat == 1 and c != 7):
            continue
        p = a = pat
        k = 0 if pat == 0 else 15
        I = 32 * c + 2 * k + p
        rely_inv = I + 0.5 - 128.0
        rely_int = 1.5 - p - 2 * a
        for ri, (q, b) in enumerate([(0, 0), (0, 1), (1, 0), (1, 1)]):
            relx_int = 1.5 - q - 2 * b
            J = 2 * np.arange(128, dtype=np.float32) + q
            relx_inv = J + 0.5 - 128.0
            d = (rely_inv - rely_int) * W1y[:, None] + \
                np.outer(W1x, relx_inv - relx_int)  # [256, 128]
            base0 = ((pat * 4 + ri) * 2 + 0) * 128
            base1 = ((pat * 4 + ri) * 2 + 1) * 128
            rowfix[:, base0:base0 + 128] = d[:128]
            rowfix[:, base1:base1 + 128] = d[128:]

    w1 = np.stack([W1[0:128], W1[128:256], W1[256:384]], axis=1).reshape(128, 768)
    w2 = np.stack([W2[0:128], W2[128:256]], axis=1).reshape(128, 256)
    b2sb = np.ascontiguousarray(b2[:, None])
    b3sb = np.zeros((128, 1), np.float32)
    b3sb[:32, 0] = b3
    selS = np.zeros((27, 9), np.float32)
    for g9 in range(9):
        for cch in range(3):
            selS[3 * g9 + cch, g9] = 1.0
    selB = np.zeros((9, 4), np.float32)
    for p in range(2):
        for q in range(2):
            for a in range(2):
                for b in range(2):
                    selB[3 * (p + a) + (q + b), 2 * p + q] += 1.0
    selR = np.zeros((128, 32), np.float32)
    for j in range(4):
        selR[32 * j + np.arange(32), np.arange(32)] = 1.0

    return {
        "fc0": fc0, "fc1": fc1, "guide": np.ascontiguousarray(g.reshape(128, 8192)),
        "w1": np.ascontiguousarray(w1), "w2": np.ascontiguousarray(w2),
        "w3": np.ascontiguousarray(W3), "bias1": bias1, "b2": b2sb, "b3": b3sb,
        "colfix": colfix, "rowfix": rowfix,
        "selS": selS, "selB": selB, "selR": selR,
    }


def kernel(**inputs):
    feat = np.asarray(inputs["feat"], np.float32)
    lr_guide = np.asarray(inputs["lr_guide"], np.float32)
    hr_guide = np.asarray(inputs["hr_guide"], np.float32)
    W1 = np.asarray(inputs["W1"], np.float32)
    b1 = np.asarray(inputs["b1"], np.float32)
    W2 = np.asarray(inputs["W2"], np.float32)
    b2 = np.asarray(inputs["b2"], np.float32)
    W3 = np.asarray(inputs["W3"], np.float32)
    b3 = np.asarray(inputs["b3"], np.float32)

    nc = _build_nc()
    in_maps = [_prep_core(c, feat, lr_guide, hr_guide, W1, b1, W2, b2, W3, b3)
               for c in range(NCORES)]
    res = run_bass_kernel_spmd(nc, in_maps, core_ids=list(range(NCORES)))
    out = np.zeros((1, 32, 256, 256), np.float32)
    for c in range(NCORES):
        yc = res.results[c]["y"].reshape(32, 4, 16, 128)
        strip = out[0, :, 32 * c:32 * c + 32, :]
        for p in range(2):
            for q in range(2):
                strip[:, p::2, q::2] = yc[:, 2 * p + q]
    return out

